# revision 21
# baseline (speedup 1.0000x reference)
"""Masked dot-product attention on 8 Trainium2 NeuronCores (Bass/Tile), v2.

Problem: B=8, H=16, S=1024, D=64 attention where scores at key positions
k >= valid_lens[b] are masked to 1e-6 (not -inf) before softmax.

Sharding (SPMD, one program on 8 cores): core m takes heads (b, 2m+j) for
all batches b, j in {0,1} - 16 head slots per core, identical workload
vector across cores; program specialized to cvec (compile cached).

v2 design (vs v1: see kernel_v1.py docstring):
  - All inputs packed on HOST into ONE bf16 dram blob per core, laid out so
    each head slot is a single contiguous-row DMA ([128, 512+C*193] slice):
    qt rows = q halves stacked (qsplit), kt rows = kT duplicated across both
    partition halves, va rows = V_aug chunks (ones column + fold).
  - QK per 128-key chunk: 2 row-tiled bf16 matmuls -> scores PSUM [128,1024].
  - exp: most chunks on ACT (exp(s/8), 1 elem/lane/cyc); every `dve_every`-th
    chunk on the DVE via 2 custom ops: EXPQ_ANT (deg-4 poly ~ exp(s/128),
    p(0)=1 exactly) then POW16_ANT (4 squarings) - rel err ~3e-4. This splits
    the exp bottleneck across two engines.
  - PV per chunk: 8 q-tile matmuls with pt[:, j*128:(j+1)*128] as bf16
    weights, rhs = V_aug chunk [128, 65] -> out PSUM [128 q, j, 65] directly
    in [q, d] layout (no PE transposes, no PSUM evacuation copies). Column 64
    accumulates the softmax denominator (ones column of V_aug).
  - epilogue: RECIPROCAL_APPROX_FAST on the 8 denominators per q-partition,
    one broadcast tensor_mul PSUM->SBUF (bf16), one SWDGE output DMA.
  - Output is bf16 [128, 8, 64] per slot, host-unpacked to fp32 [S, D].
"""

from contextlib import ExitStack

import numpy as np
import ml_dtypes

import concourse.bass as bass  # noqa: F401
import concourse.mybir as mybir
import concourse.tile as tile
from concourse import bacc

F32 = mybir.dt.float32
BF16 = mybir.dt.bfloat16
NP_BF16 = ml_dtypes.bfloat16

B, H, S, D = 8, 16, 1024, 64
N_CORES = 8
HPC = H // N_CORES     # heads per (core, batch) = 2
KC = S // 128          # key chunks per full head
EXPF = mybir.ActivationFunctionType.Exp
SCALE = 1.0 / 8.0      # 1/sqrt(64)

DENSE_CVEC = (KC,) * B

# deg-4 p(y) ~= e^y on [-0.5,0.5] with p(0)=1 (fit_poly.py), in raw-score
# domain y = s/128: exp(s/8) = p(s)^16, rel err ~2.9e-4.
PC1 = 0.0078114615999729945
PC2 = 3.052577174400179e-05
PC3 = 8.06471076669409e-08
PC4 = 1.539516309353584e-10

CFG = {
    "lag": 5,          # chunks between exp and its PV consumption
    "dve_mod": 4,      # DVE-chunk cadence: gidx % dve_mod in dve_res
    "dve_res": (1,),
    "act_tail": 3,     # last n chunks never go to the DVE (endgame drain)
    "qk_pair": False,   # at DVE chunks, emit next chunk's QK first so the
                       # ps_s double-buffer wait lands on the DVE, not ACT
    "in_bufs": 6,
    "pt_bufs": 10,
    "tt_bufs": 3,
    "ob_bufs": 4,
    "ps_s_bufs": 3,
    "ps_o_bufs": 1,
    "in_ring": "sp",   # input DMA ring: sp | act | pool
    "out_ring": "pool",  # output DMA ring: pool (SWDGE) | sp | act
}

# ---------------------------------------------------------------------------
# Custom DVE ops: EXPQ_ANT (poly4, 8 ALU stages) and POW16_ANT (4 squarings).
# Registered at import into concourse.dve_ops' tables (runtime append).
# ---------------------------------------------------------------------------
_DVE_OPS = {}


def _register_dve_ops():
    if _DVE_OPS:
        return _DVE_OPS
    import concourse.dve_ops as dve_ops
    from concourse.dve_ops import DveOp, OPS, CUSTOM_DVE_SPECS, _SUB_OPCODE_FOR_NAME
    from concourse.dve_spec import (
        Spec, Src0, C0, C1, C2, C3, One, lower, sq, _spill_c3_to_src1, _has_src1,
    )
    from concourse.dve_uop import DveOpSpec

    def _ref_expq(in0, in1, s0, s1, imm2):
        x = np.asarray(in0, np.float32)
        t = (np.float32(s0) * x + np.float32(s1)).astype(np.float32)
        t = (t * x + np.float32(imm2)).astype(np.float32)
        t = (t * x + np.asarray(in1, np.float32)).astype(np.float32)
        return (t * x + np.float32(1.0)).astype(np.float32)

    def _ref_pow16(in0, in1, s0, s1, imm2):
        t = np.asarray(in0, np.float32)
        for _ in range(4):
            t = (t * t).astype(np.float32)
        return t

    # p(x) = (((C0*x + C1)*x + C2)*x + C3)*x + 1 ; C3 spilled to Src1.
    _expq_body = _spill_c3_to_src1(
        (((Src0 * C0 + C1) * Src0 + C2) * Src0 + C3) * Src0 + One
    )
    specs = {
        "EXPQ_ANT": (Spec(body=_expq_body, reference=_ref_expq), ),
        "POW16_ANT": (Spec(body=sq(sq(sq(sq(Src0)))), reference=_ref_pow16), ),
    }

    existing = {op.name for op in OPS}
    for name, (spec,) in specs.items():
        if name in existing:
            _DVE_OPS[name] = next(op for op in OPS if op.name == name)
            continue
        row = max(_SUB_OPCODE_FOR_NAME.values()) + 1
        assert row < 0x20, "custom-DVE row budget exhausted"
        _SUB_OPCODE_FOR_NAME[name] = row
        CUSTOM_DVE_SPECS[name] = spec
        shas = {}
        for ver in ("v3", "v4"):
            try:
                tmp = DveOpSpec(name=name, opcode=row, uops=lower(spec, ver=ver),
                                rd1_en=_has_src1(spec))
                shas[ver] = tmp.sha(ver)
            except Exception:
                pass
        op = DveOp(name, spec, subdim=False, uops_sha=shas)
        OPS.append(op)
        _DVE_OPS[name] = op
    return _DVE_OPS


# ---------------------------------------------------------------------------
# Slot planning (order heads so heavy/light interleave; identical to v1).
# ---------------------------------------------------------------------------

def cvec_of(valid_lens):
    vl = np.asarray(valid_lens).astype(np.int64).reshape(B)
    return tuple(int(min(KC, L // 128 + 1)) for L in vl)


def slot_plan(cvec):
    pairs = sorted([(cvec[b], b) for b in range(B) for _ in range(HPC)],
                   key=lambda x: (-x[0], x[1]))
    last = pairs.pop()[1]
    first = pairs.pop()[1]
    order = [first]
    lo, hi = 0, len(pairs) - 1
    while lo <= hi:
        order.append(pairs[lo][1])
        lo += 1
        if lo <= hi:
            order.append(pairs[hi][1])
            hi -= 1
    order.append(last)
    return order


def slot_heads(cvec):
    """[(batch, j)] per slot; head of slot s on core m is (batch, 2m + j)."""
    plan = slot_plan(cvec)
    occ = {}
    out = []
    for b in plan:
        j = occ.get(b, 0)
        occ[b] = j + 1
        out.append((b, j))
    return out


def slot_layout(cvec):
    """Per-slot (batch, j, C, in_off, in_w). in widths in bf16 elements."""
    rows = []
    off = 0
    for (b, j) in slot_heads(cvec):
        C = cvec[b]
        w = 512 + C * 128 + C * 65
        rows.append((b, j, C, off, w))
        off += w
    return rows, off


# ---------------------------------------------------------------------------
# Device program
# ---------------------------------------------------------------------------

def _in_eng(nc):
    return {"sp": nc.sync, "act": nc.scalar, "pool": nc.gpsimd}[CFG["in_ring"]]


def _out_eng(nc):
    return {"sp": nc.sync, "act": nc.scalar, "pool": nc.gpsimd}[CFG["out_ring"]]


def build_program(cvec=DENSE_CVEC, loop: int = 1):
    ops = _register_dve_ops()
    from concourse.dve_ops import RECIPROCAL_APPROX_FAST, RECIP_APPROX_FAST_CONSTS

    layout, tot_in = slot_layout(cvec)

    nc = bacc.Bacc("TRN2", target_bir_lowering=False, debug=False,
                   enable_asserts=True, num_devices=N_CORES)
    inp = nc.dram_tensor("inp", [128, tot_in], BF16, kind="ExternalInput").ap()
    outp = nc.dram_tensor("out", [128, H * 512], BF16,
                          kind="ExternalOutput").ap()

    with tile.TileContext(nc) as tc:
        with ExitStack() as ctx:
            const_pool = ctx.enter_context(tc.tile_pool(name="const", bufs=1))
            c3t = const_pool.tile([128, 1], F32)   # spilled poly coeff (PC1)
            nc.vector.memset(c3t[:], PC1)
            # Trigger the exp ACT-table load (~2.7us) immediately so it
            # overlaps the first input DMA instead of the first real exp.
            warm = const_pool.tile([128, 1], F32)
            nc.scalar.activation(warm[:], c3t[:], EXPF, scale=SCALE)

            in_pool = ctx.enter_context(tc.tile_pool(name="in", bufs=CFG["in_bufs"]))
            pt_pool = ctx.enter_context(tc.tile_pool(name="pt", bufs=CFG["pt_bufs"]))
            tt_pool = ctx.enter_context(tc.tile_pool(name="tt", bufs=CFG["tt_bufs"]))
            ob_pool = ctx.enter_context(tc.tile_pool(name="ob", bufs=CFG["ob_bufs"]))
            sc_pool = ctx.enter_context(tc.tile_pool(name="sc", bufs=4))
            ps_s_pool = ctx.enter_context(
                tc.tile_pool(name="ps_s", bufs=CFG["ps_s_bufs"], space="PSUM"))
            ps_o_pool = ctx.enter_context(
                tc.tile_pool(name="ps_o", bufs=CFG["ps_o_bufs"], space="PSUM"))

            total_chunks = sum(C for (_b, _j, C, _o, _w) in layout)
            gidx = [0]   # global chunk counter for ACT/DVE assignment
            # Flat cross-head pipeline: ("pv", fn) entries are flushed `lag`
            # chunks behind their exp; ("epi", fn) entries ride the queue
            # right after their head's last pv (not counted toward the lag).
            pvq = []

            def pv_depth():
                return sum(1 for kind, _fn in pvq if kind == "pv")

            def flush_one():
                pvq.pop(0)[1]()

            def emit_slot(s, b, j, C, in_off, in_w):
                in_t = in_pool.tile([128, in_w], BF16, tag="in")
                _in_eng(nc).dma_start(in_t[:], inp[:, in_off:in_off + in_w])
                qt = in_t[:, 0:512]
                kt = in_t[:, 512:512 + C * 128]
                va = in_t[:, 512 + C * 128:]

                ps_o = ps_o_pool.tile([128, 8, 128], F32, tag="ps_o")

                def make_pv(kc, pt):
                    def emit_pv():
                        # ps_o spans 2 PSUM banks (4 q-tiles each).
                        # start=True zeroes a whole 2KB bank, so only the
                        # first q-tile per bank starts; the rest land on
                        # has_written=0 elements (overwrite mode).
                        for q in range(8):
                            nc.tensor.matmul(
                                ps_o[:, q, 0:D + 1],
                                lhsT=pt[:, q * 128:(q + 1) * 128],
                                rhs=va[:, kc * 65:(kc + 1) * 65],
                                start=(kc == 0 and q % 4 == 0),
                                stop=(kc == C - 1 and q % 4 == 3),
                            )
                    return emit_pv

                def epilogue():
                    rec = sc_pool.tile([128, 8], F32, tag="rec")
                    nc.vector._custom_dve(
                        RECIPROCAL_APPROX_FAST, out=rec[:], in0=ps_o[:, :, D],
                        **RECIP_APPROX_FAST_CONSTS)
                    ob = ob_pool.tile([128, 8, D], BF16, tag="ob")
                    nc.vector.tensor_mul(
                        ob[:], ps_o[:, :, 0:D],
                        rec[:].unsqueeze(2).broadcast_to((128, 8, D)))
                    _out_eng(nc).dma_start(
                        outp[:, s * 512:(s + 1) * 512].rearrange(
                            "p (j d) -> p j d", j=8),
                        ob[:])

                def emit_qk(kc):
                    ps_s = ps_s_pool.tile([128, S], F32, tag="ps_s")
                    nc.tensor.matmul(
                        ps_s[:, 0:512],
                        lhsT=kt[0:64, kc * 128:(kc + 1) * 128],
                        rhs=qt[0:64, :], start=True, stop=True)
                    nc.tensor.matmul(
                        ps_s[:, 512:1024],
                        lhsT=kt[64:128, kc * 128:(kc + 1) * 128],
                        rhs=qt[64:128, :], start=True, stop=True)
                    return ps_s

                def emit_exp_act(kc, ps_s):
                    pt = pt_pool.tile([128, S], BF16, tag="pt")
                    nc.scalar.activation(pt[:], ps_s[:], EXPF, scale=SCALE)
                    return pt

                def emit_exp_dve(kc, ps_s):
                    pt = pt_pool.tile([128, S], BF16, tag="pt")
                    tt = tt_pool.tile([128, S], F32, tag="tt")
                    nc.vector._custom_dve(
                        ops["EXPQ_ANT"], out=tt[:], in0=ps_s[:],
                        in1=c3t[:], s0=PC4, s1=PC3, imm2=PC2)
                    nc.vector._custom_dve(
                        ops["POW16_ANT"], out=pt[:], in0=tt[:])
                    return pt

                def push(kc, pt):
                    pvq.append(("pv", make_pv(kc, pt)))
                    if kc == C - 1:
                        pvq.append(("epi", epilogue))
                    while pv_depth() > CFG["lag"]:
                        flush_one()

                kc = 0
                while kc < C:
                    use_dve = (gidx[0] % CFG["dve_mod"] in CFG["dve_res"]
                               and gidx[0] < total_chunks - CFG["act_tail"])
                    if use_dve and CFG["qk_pair"] and kc + 1 < C:
                        ps_b = emit_qk(kc + 1)   # next (ACT) chunk's QK first
                        ps_a = emit_qk(kc)
                        pt_a = emit_exp_dve(kc, ps_a)
                        pt_b = emit_exp_act(kc + 1, ps_b)
                        gidx[0] += 2
                        push(kc, pt_a)
                        push(kc + 1, pt_b)
                        kc += 2
                        continue
                    ps_s = emit_qk(kc)
                    if use_dve:
                        pt = emit_exp_dve(kc, ps_s)
                    else:
                        pt = emit_exp_act(kc, ps_s)
                    gidx[0] += 1
                    push(kc, pt)
                    kc += 1

            def body(_i=None):
                for s, (b, j, C, in_off, in_w) in enumerate(layout):
                    emit_slot(s, b, j, C, in_off, in_w)
                while pvq:
                    flush_one()

            if loop == 1:
                body()
            else:
                with tc.For_i(0, loop, 1):
                    body()
    nc.compile()
    return nc


# ---------------------------------------------------------------------------
# Host packing / unpacking
# ---------------------------------------------------------------------------

def make_in_maps(queries, keys, values, valid_lens):
    q = np.ascontiguousarray(
        np.asarray(queries, dtype=np.float32)).reshape(B, H, S, D)
    k = np.ascontiguousarray(
        np.asarray(keys, dtype=np.float32)).reshape(B, H, S, D)
    v = np.ascontiguousarray(
        np.asarray(values, dtype=np.float32)).reshape(B, H, S, D)
    vl = np.asarray(valid_lens).astype(np.int64).reshape(B)
    cvec = cvec_of(vl)
    layout, tot_in = slot_layout(cvec)

    km = k.copy()
    va = np.empty((B, H, S, D + 1), np.float32)
    va[..., :D] = v
    va[..., D] = 1.0
    for b in range(B):
        L, C = int(vl[b]), cvec[b]
        km[b, :, L:, :] = 0.0
        if C < KC:
            va[b, :, C * 128 - 1, :] += va[b, :, C * 128:, :].sum(axis=1)

    qT = np.ascontiguousarray(q.transpose(0, 1, 3, 2))   # [B, H, D, S]
    kT = np.ascontiguousarray(km.transpose(0, 1, 3, 2))

    in_maps = []
    for m in range(N_CORES):
        blob = np.empty((128, tot_in), NP_BF16)
        for (b, j, C, off, w) in layout:
            h = 2 * m + j
            qt = blob[:, off:off + 512]
            qt[0:64] = qT[b, h][:, 0:512]
            qt[64:128] = qT[b, h][:, 512:1024]
            kt = blob[:, off + 512:off + 512 + C * 128]
            kt[0:64] = kT[b, h][:, 0:C * 128]
            kt[64:128] = kT[b, h][:, 0:C * 128]
            vt = blob[:, off + 512 + C * 128:off + w]
            vt[:] = va[b, h, 0:C * 128].reshape(C, 128, D + 1).transpose(
                1, 0, 2).reshape(128, C * 65)
        in_maps.append({"inp": blob})
    return in_maps, cvec


def unpack_core_out(arr):
    """[128, 8192] bf16 -> [16 slots, S, D] fp32."""
    a = np.asarray(arr)
    if a.dtype != NP_BF16:
        a = a.view(NP_BF16)
    return (a.reshape(128, H, 8, D).transpose(1, 2, 0, 3)
            .reshape(H, S, D).astype(np.float32))


def scatter_outputs(results, cvec):
    out = np.empty((B, H, S, D), dtype=np.float32)
    for m in range(N_CORES):
        so = unpack_core_out(results[m])
        for s, (b, j) in enumerate(slot_heads(cvec)):
            out[b, 2 * m + j] = so[s]
    return out.reshape(B * H, S, D)


_NC_CACHE = {}


def _get_nc(cvec, loop=1):
    key = (cvec, loop, tuple(sorted(CFG.items())))
    if key not in _NC_CACHE:
        _NC_CACHE[key] = build_program(cvec, loop)
    return _NC_CACHE[key]


def kernel(queries, keys, values, valid_lens):
    from concourse.bass_utils import run_bass_kernel_spmd

    in_maps, cvec = make_in_maps(queries, keys, values, valid_lens)
    nc = _get_nc(cvec)
    res = run_bass_kernel_spmd(nc, in_maps, list(range(N_CORES)))
    return scatter_outputs(
        [res.results[m]["out"] for m in range(N_CORES)], cvec)


# ----------------------------------------------------------------------------
# Cached jitted runner (used by test.py for timing; avoids per-call re-trace
# and ships inputs to the devices once).
# ----------------------------------------------------------------------------
_RUNNER_CACHE = {}


def _get_runner(cvec=DENSE_CVEC, loop: int = 1):
    key = (cvec, loop, tuple(sorted(CFG.items())))
    if key in _RUNNER_CACHE:
        return _RUNNER_CACHE[key]

    import jax
    from jax.sharding import Mesh, PartitionSpec, NamedSharding
    from jax.experimental.shard_map import shard_map
    from concourse import bass2jax

    nc = _get_nc(cvec, loop)
    bass2jax.install_neuronx_cc_hook()

    partition_name = (nc.partition_id_tensor.name
                      if nc.partition_id_tensor else None)
    in_names, out_names, out_avals, zero_outs = [], [], [], []
    for alloc in nc.m.functions[0].allocations:
        if not isinstance(alloc, mybir.MemoryLocationSet):
            continue
        name = alloc.memorylocations[0].name
        if alloc.kind == "ExternalInput":
            if name != partition_name:
                in_names.append(name)
        elif alloc.kind == "ExternalOutput":
            out_names.append(name)
            shape = tuple(alloc.tensor_shape)
            dtype = mybir.dt.np(alloc.dtype)
            out_avals.append(jax.core.ShapedArray(shape, dtype))
            zero_outs.append(np.zeros(shape, dtype))
    n_params = len(in_names)
    n_outs = len(out_avals)
    all_in_names = in_names + out_names
    if partition_name is not None:
        all_in_names = all_in_names + [partition_name]

    def _body(*args):
        operands = list(args)
        if partition_name is not None:
            operands.append(bass2jax.partition_id_tensor())
        outs = bass2jax._bass_exec_p.bind(
            *operands,
            out_avals=tuple(out_avals),
            in_names=tuple(all_in_names),
            out_names=tuple(out_names),
            lowering_input_output_aliases=(),
            sim_require_finite=True,
            sim_require_nnan=True,
            nc=nc,
        )
        return tuple(outs)

    devices = jax.devices()[:N_CORES]
    mesh = Mesh(np.asarray(devices), ("core",))
    donate = tuple(range(n_params, n_params + n_outs))
    sharded = jax.jit(
        shard_map(
            _body, mesh=mesh,
            in_specs=(PartitionSpec("core"),) * (n_params + n_outs),
            out_specs=(PartitionSpec("core"),) * n_outs,
            check_rep=False,
        ),
        donate_argnums=donate, keep_unused=True,
    )

    def run(in_maps):
        concat_in = [
            np.concatenate([m[name] for m in in_maps], axis=0)
            for name in in_names
        ]
        concat_zeros = [
            np.zeros((N_CORES * z.shape[0], *z.shape[1:]), z.dtype)
            for z in zero_outs
        ]
        out_arrs = sharded(*concat_in, *concat_zeros)
        return [
            {
                name: np.asarray(out_arrs[i]).reshape(
                    N_CORES, *out_avals[i].shape)[c]
                for i, name in enumerate(out_names)
            }
            for c in range(N_CORES)
        ]

    def make_dev_args(in_maps):
        sh = NamedSharding(mesh, PartitionSpec("core"))
        concat_in = [
            np.concatenate([m[name] for m in in_maps], axis=0)
            for name in in_names
        ]
        dev_in = [jax.device_put(a, sh) for a in concat_in]
        jax.block_until_ready(dev_in)

        def fresh_zeros():
            zs = [jax.device_put(
                np.zeros((N_CORES * z.shape[0], *z.shape[1:]), z.dtype), sh)
                for z in zero_outs]
            jax.block_until_ready(zs)
            return zs

        return dev_in, fresh_zeros

    _RUNNER_CACHE[key] = (run, sharded, make_dev_args, out_names, out_avals, nc)
    return _RUNNER_CACHE[key]
